# revision 1
# baseline (speedup 1.0000x reference)
"""Trainium2 kernel for DigitConvolutionalModel.

Model: x(B,784) -> reshape(28,28) -> conv3x3 'VALID' (cross-correlation)
       -> flatten(676) -> Linear(676,256)+ReLU -> Linear(256,10).

The conv is linear, so it folds into the first Linear:
    feat = x @ Wc          (Wc: 784x676 sparse conv matrix)
    h    = relu(feat @ w1 + b1) = relu(x @ (Wc @ w1) + b1)
Device work is then two GEMMs per batch tile:
    H^T = relu(W_eff^T-tiles . x^T + b1);  out^T = w2^T . H + b2

Sharding: pure data parallel over 8 cores (8192 rows each). The host
pre-transposes each shard to x^T (contraction dim on SBUF partitions) and
casts to bf16 so the PE streams it directly; weights are replicated and
pre-packed so all constants arrive in 4 DMAs.
"""

import os
from contextlib import ExitStack

import numpy as np
import ml_dtypes

import concourse.bass as bass
import concourse.tile as tile
from concourse import bacc, mybir
from concourse.bass_utils import run_bass_kernel_spmd

N_CORES = 8
B = 65536
B_SHARD = B // N_CORES  # 8192
K = 784                 # contraction dim (pixels)
KT = 112                # k-tile partition size (7 * 112 = 784)
NKT = K // KT
CH = 256                # hidden channels
MT = 128                # m-tile (output channels per matmul)
NMT = CH // MT
OUT_CH = 10
OUT_PAD = 16            # padded output channels
CHUNK = 1024            # batch columns per x DMA
SUB = 512               # matmul moving free dim / PSUM bank
OGRP = 2048             # output store granularity (batch columns)
BF16 = mybir.dt.bfloat16
F32 = mybir.dt.float32

_CACHE: dict = {}


def _build(b_shard: int):
    nc = bacc.Bacc(
        "TRN2",
        target_bir_lowering=False,
        debug=False,
        num_devices=N_CORES,
    )
    xT = nc.dram_tensor("xT", [K, b_shard], BF16, kind="ExternalInput")
    # GEMM1 weight tiles packed side by side, m-major: [112, (m*NKT+t)*MT + j]
    wta = nc.dram_tensor("wta", [KT, NKT * NMT * MT], BF16, kind="ExternalInput")
    b1a = nc.dram_tensor("b1a", [MT, NMT], F32, kind="ExternalInput")
    w2a = nc.dram_tensor("w2a", [MT, NMT * OUT_PAD], BF16, kind="ExternalInput")
    b2c = nc.dram_tensor("b2c", [OUT_PAD, 1], F32, kind="ExternalInput")
    outT = nc.dram_tensor("outT", [OUT_PAD, b_shard], F32, kind="ExternalOutput")

    relu = mybir.ActivationFunctionType.Relu
    ident = mybir.ActivationFunctionType.Identity
    chunks = [512, 512] + [CHUNK] * ((b_shard - 1024) // CHUNK)
    assert sum(chunks) == b_shard
    n_ogrp = b_shard // OGRP

    with tile.TileContext(nc) as tc, ExitStack() as ctx:
        const = ctx.enter_context(tc.tile_pool(name="const", bufs=1))
        # GEMM2 constants in their own pool: sharing the bufs=1 const pool
        # with the GEMM1 weights trips a scheduler slot-wait deadlock.
        const2 = ctx.enter_context(tc.tile_pool(name="const2", bufs=1))
        opool = ctx.enter_context(tc.tile_pool(name="out", bufs=1))
        xpool = ctx.enter_context(tc.tile_pool(name="xin", bufs=4))
        hpool = ctx.enter_context(tc.tile_pool(name="h", bufs=4))
        hps = ctx.enter_context(
            tc.tile_pool(name="hps", bufs=2, space=bass.MemorySpace.PSUM)
        )
        ops = ctx.enter_context(
            tc.tile_pool(name="ops", bufs=2, space=bass.MemorySpace.PSUM)
        )

        # --- resident weights/biases, on the ACT ring.  The m=0 weight
        # half loads first so the first PSUM group can start sooner. ---
        HW = NKT * MT
        b1_all = const.tile([MT, NMT], F32, tag="b1a")
        nc.scalar.dma_start(b1_all[:], b1a[:, :])
        wt_m = []
        for m in range(NMT):
            wtile = const.tile([KT, HW], BF16, tag=f"wta{m}", name=f"wt_m{m}")
            nc.scalar.dma_start(wtile[:], wta[:, m * HW:(m + 1) * HW])
            wt_m.append(wtile)
        w2_all = const2.tile([MT, NMT * OUT_PAD], BF16, tag="w2a")
        nc.scalar.dma_start(w2_all[:], w2a[:, :])
        b2_sb = const2.tile([OUT_PAD, 1], F32, tag="b2")
        nc.scalar.dma_start(b2_sb[:], b2c[:, :])

        def w_sb(t, m):
            return wt_m[m][:, t * MT:(t + 1) * MT]

        # Output accumulates in SBUF, streamed out in OGRP slabs on the
        # GpSimd (SWDGE) ring so stores overlap compute and never queue
        # behind x prefetch loads (FIFO slot-wait deadlock).
        oall = [
            opool.tile([OUT_PAD, OGRP], F32, tag=f"o{g}", name=f"oall{g}")
            for g in range(n_ogrp)
        ]

        # GEMM2 runs one chunk behind GEMM1 (software pipeline): by the
        # time it streams h, the relu that produced h is long done, so the
        # PE never stalls on the ACT semaphore.
        pending = []  # [(hb0, hb1, j0)] sub-blocks awaiting GEMM2

        def flush_gemm2():
            while pending:
                hb2, j0 = pending.pop(0)
                po = ops.tile([OUT_PAD, SUB], F32, tag="po", name="po")
                for m in range(NMT):
                    nc.tensor.matmul(
                        po[:],
                        w2_all[:, m * OUT_PAD:(m + 1) * OUT_PAD],
                        hb2[m][:],
                        start=(m == 0),
                        stop=(m == NMT - 1),
                    )
                g = j0 // OGRP
                nc.vector.tensor_scalar_add(
                    oall[g][:, j0 - g * OGRP:j0 - g * OGRP + SUB],
                    po[:], b2_sb[:],
                )
                if (j0 + SUB) % OGRP == 0:
                    nc.sync.dma_start(
                        outT[:, g * OGRP:(g + 1) * OGRP], oall[g][:]
                    )

        # --- main loop over batch chunks ---
        coff = 0
        for c, csz in enumerate(chunks):
            xt = []
            for t in range(NKT):
                xtile = xpool.tile([KT, csz], BF16, tag=f"x{t}")
                nc.sync.dma_start(
                    xtile[:], xT[t * KT:(t + 1) * KT, coff:coff + csz]
                )
                xt.append(xtile)
            for s in range(csz // SUB):
                hb = []
                for m in range(NMT):
                    ps = hps.tile([MT, SUB], F32, tag=f"ps{m}")
                    for t in range(NKT):
                        nc.tensor.matmul(
                            ps[:],
                            w_sb(t, m),
                            xt[t][:, s * SUB:(s + 1) * SUB],
                            start=(t == 0),
                            stop=(t == NKT - 1),
                        )
                    h = hpool.tile([MT, SUB], BF16, tag=f"h{m}")
                    nc.scalar.activation(h[:], ps[:], relu, bias=b1_all[:, m:m + 1])
                    hb.append(h)
                    if s == 0 and m == 0:
                        flush_gemm2()  # previous chunk's GEMM2, relus ready
                pending.append((hb, coff + s * SUB))
            coff += csz
        flush_gemm2()

    nc.compile()
    return nc


def _get_nc(b_shard: int = B_SHARD):
    if b_shard not in _CACHE:
        _CACHE[b_shard] = _build(b_shard)
    return _CACHE[b_shard]


def _host_prep(x, w_conv, w1, b1, w2, b2, b_shard=B_SHARD):
    """Fold conv into w1, pack weights, and lay out per-core inputs."""
    bf16 = ml_dtypes.bfloat16
    # Conv matrix Wc[784, 676]: feat[:, oi*26+oj] = sum_{di,dj} x[:, (oi+di)*28+(oj+dj)] * w_conv[di,dj]
    w_conv = np.asarray(w_conv, np.float64)
    oi = np.arange(26)
    oj = np.arange(26)
    wc = np.zeros((784, 676), np.float64)
    for di in range(3):
        for dj in range(3):
            src = ((oi[:, None] + di) * 28 + (oj[None, :] + dj)).ravel()
            dst = (oi[:, None] * 26 + oj[None, :]).ravel()
            wc[src, dst] += w_conv[di, dj]
    w_eff = (wc @ np.asarray(w1, np.float64)).astype(bf16)  # [784, 256]

    # wta[p, (m*NKT+t)*MT + j] = w_eff[t*KT+p, m*MT+j]  (m-major)
    wta = np.ascontiguousarray(
        w_eff.reshape(NKT, KT, NMT, MT).transpose(1, 2, 0, 3).reshape(KT, -1)
    )
    # b1a[p, m] = b1[m*MT+p]
    b1a = np.ascontiguousarray(
        np.asarray(b1, np.float32).reshape(NMT, MT).T
    )
    # w2a[p, m*OUT_PAD + j] = w2_padded[m*MT+p, j]
    w2p = np.zeros((CH, OUT_PAD), bf16)
    w2p[:, :OUT_CH] = np.asarray(w2).astype(bf16)
    w2a = np.ascontiguousarray(
        w2p.reshape(NMT, MT, OUT_PAD).transpose(1, 0, 2).reshape(MT, -1)
    )
    b2c = np.zeros((OUT_PAD, 1), np.float32)
    b2c[:OUT_CH, 0] = np.asarray(b2, np.float32)

    x_bf = np.asarray(x).astype(bf16)  # [B, 784]
    in_maps = []
    for c in range(N_CORES):
        shard = x_bf[c * b_shard:(c + 1) * b_shard]
        in_maps.append(
            {
                "xT": np.ascontiguousarray(shard.T),  # [784, b_shard]
                "wta": wta,
                "b1a": b1a,
                "w2a": w2a,
                "b2c": b2c,
            }
        )
    return in_maps


LAST_RESULT = None  # BassKernelResults of the most recent run (for test harness)


def kernel(x, w_conv, w1, b1, w2, b2):
    global LAST_RESULT
    nc = _get_nc()
    in_maps = _host_prep(x, w_conv, w1, b1, w2, b2)
    trace = bool(int(os.environ.get("KERNEL_TRACE", "0")))
    res = run_bass_kernel_spmd(
        nc, in_maps, list(range(N_CORES)), trace=trace,
        tmpdir=os.environ.get("KERNEL_TMPDIR") or None,
    )
    LAST_RESULT = res
    out = np.empty((B, OUT_CH), np.float32)
    for c in range(N_CORES):
        out[c * B_SHARD:(c + 1) * B_SHARD] = res.results[c]["outT"][:OUT_CH].T
    return out



# revision 2
# speedup vs baseline: 1.0631x; 1.0631x over previous
"""Trainium2 kernel for DigitConvolutionalModel.

Model: x(B,784) -> reshape(28,28) -> conv3x3 'VALID' (cross-correlation)
       -> flatten(676) -> Linear(676,256)+ReLU -> Linear(256,10).

The conv is linear, so it folds into the first Linear:
    feat = x @ Wc          (Wc: 784x676 sparse conv matrix)
    h    = relu(feat @ w1 + b1) = relu(x @ (Wc @ w1) + b1)
Device work is then two GEMMs per batch tile:
    H^T = relu(W_eff^T-tiles . x^T + b1);  out^T = w2^T . H + b2

Sharding: pure data parallel over 8 cores (8192 rows each). The host
pre-transposes each shard to x^T (contraction dim on SBUF partitions),
casts to bf16, and lays the columns out chunk-major so each steady-state
chunk arrives in ONE large DMA. Weights are replicated; the first weight
tile is tiny so the PE starts within ~1us.
"""

import os
from contextlib import ExitStack

import numpy as np
import ml_dtypes

import concourse.bass as bass
import concourse.tile as tile
from concourse import bacc, mybir
from concourse.bass_utils import run_bass_kernel_spmd

N_CORES = 8
B = 65536
B_SHARD = B // N_CORES  # 8192
K = 784                 # contraction dim (pixels)
KT = 112                # k-tile partition size (7 * 112 = 784)
NKT = K // KT
CH = 256                # hidden channels
MT = 128                # m-tile (output channels per matmul)
NMT = CH // MT
OUT_CH = 10
OUT_PAD = 16            # padded output channels
SUB = 512               # matmul moving free dim / PSUM bank
OGRP = 2048             # output store granularity (batch columns)
CHUNKS = [512, 512] + [1024] * 7  # batch columns per x DMA
BF16 = mybir.dt.bfloat16
F32 = mybir.dt.float32

_CACHE: dict = {}


def _build(b_shard: int):
    nc = bacc.Bacc(
        "TRN2",
        target_bir_lowering=False,
        debug=False,
        num_devices=N_CORES,
    )
    # x columns chunk-major: chunk c at cols [7*off, 7*(off+csz)), inside
    # which tile t occupies [7*off + t*csz, 7*off + (t+1)*csz).
    xH = nc.dram_tensor("xH", [KT, NKT * b_shard], BF16, kind="ExternalInput")
    # GEMM1 weights, split so the first matmul's tile lands first:
    #   w00: (t=0, m=0);  w0r: (t=1..6, m=0);  w1a: all of m=1
    w00 = nc.dram_tensor("w00", [KT, MT], BF16, kind="ExternalInput")
    w0r = nc.dram_tensor("w0r", [KT, (NKT - 1) * MT], BF16, kind="ExternalInput")
    w1a = nc.dram_tensor("w1a", [KT, NKT * MT], BF16, kind="ExternalInput")
    b1a = nc.dram_tensor("b1a", [MT, NMT], F32, kind="ExternalInput")
    w2a = nc.dram_tensor("w2a", [MT, NMT * OUT_PAD], BF16, kind="ExternalInput")
    b2c = nc.dram_tensor("b2c", [OUT_PAD, 1], F32, kind="ExternalInput")
    outT = nc.dram_tensor("outT", [OUT_PAD, b_shard], F32, kind="ExternalOutput")

    relu = mybir.ActivationFunctionType.Relu
    assert sum(CHUNKS) == b_shard
    n_ogrp = b_shard // OGRP

    with tile.TileContext(nc) as tc, ExitStack() as ctx:
        const = ctx.enter_context(tc.tile_pool(name="const", bufs=1))
        # GEMM2 constants in their own pool: sharing the bufs=1 const pool
        # with the GEMM1 weights trips a scheduler slot-wait deadlock.
        const2 = ctx.enter_context(tc.tile_pool(name="const2", bufs=1))
        opool = ctx.enter_context(tc.tile_pool(name="out", bufs=1))
        xpool0 = ctx.enter_context(tc.tile_pool(name="xin0", bufs=1))
        xpool = ctx.enter_context(tc.tile_pool(name="xin", bufs=4))
        hpool = ctx.enter_context(tc.tile_pool(name="h", bufs=4))
        hps = ctx.enter_context(
            tc.tile_pool(name="hps", bufs=3, space=bass.MemorySpace.PSUM)
        )
        ops = ctx.enter_context(
            tc.tile_pool(name="ops", bufs=2, space=bass.MemorySpace.PSUM)
        )

        # --- resident weights/biases on the ACT ring, smallest-first so
        # the first matmul can start ASAP. ---
        w00_sb = const.tile([KT, MT], BF16, tag="w00")
        nc.scalar.dma_start(w00_sb[:], w00[:, :])
        b1_all = const.tile([MT, NMT], F32, tag="b1a")
        nc.scalar.dma_start(b1_all[:], b1a[:, :])
        w0r_sb = const.tile([KT, (NKT - 1) * MT], BF16, tag="w0r")
        nc.scalar.dma_start(w0r_sb[:], w0r[:, :])
        w1_sb = const.tile([KT, NKT * MT], BF16, tag="w1a")
        nc.scalar.dma_start(w1_sb[:], w1a[:, :])
        w2_all = const2.tile([MT, NMT * OUT_PAD], BF16, tag="w2a")
        nc.scalar.dma_start(w2_all[:], w2a[:, :])
        b2_sb = const2.tile([OUT_PAD, 1], F32, tag="b2")
        nc.scalar.dma_start(b2_sb[:], b2c[:, :])

        def w_sb(t, m):
            if m == 0:
                return w00_sb[:] if t == 0 else w0r_sb[:, (t - 1) * MT:t * MT]
            return w1_sb[:, t * MT:(t + 1) * MT]

        # Output accumulates in SBUF, streamed out in OGRP slabs on the
        # GpSimd (SWDGE) ring so stores never queue behind x prefetch.
        oall = [
            opool.tile([OUT_PAD, OGRP], F32, tag=f"o{g}", name=f"oall{g}")
            for g in range(n_ogrp)
        ]

        # GEMM2 runs one sub behind GEMM1 (software pipeline): by the time
        # it streams h, the relu that produced h is long done, so the PE
        # never stalls on the ACT semaphore.
        pending = []  # [(hb, j0)] sub-blocks awaiting GEMM2

        def flush_gemm2():
            while pending:
                hb2, j0 = pending.pop(0)
                po = ops.tile([OUT_PAD, SUB], F32, tag="po", name="po")
                for m in range(NMT):
                    nc.tensor.matmul(
                        po[:],
                        w2_all[:, m * OUT_PAD:(m + 1) * OUT_PAD],
                        hb2[m][:],
                        start=(m == 0),
                        stop=(m == NMT - 1),
                    )
                g = j0 // OGRP
                nc.vector.tensor_scalar_add(
                    oall[g][:, j0 - g * OGRP:j0 - g * OGRP + SUB],
                    po[:], b2_sb[:],
                )
                if (j0 + SUB) % OGRP == 0:
                    nc.gpsimd.dma_start(
                        outT[:, g * OGRP:(g + 1) * OGRP], oall[g][:]
                    )

        # --- main loop over batch chunks ---
        coff = 0
        for c, csz in enumerate(CHUNKS):
            hoff = NKT * coff
            if c == 0:
                # First chunk: per-tile DMAs so the first matmul only
                # waits for one small transfer.
                xt = []
                for t in range(NKT):
                    xtile = xpool0.tile([KT, csz], BF16, tag=f"x{t}")
                    nc.sync.dma_start(
                        xtile[:], xH[:, hoff + t * csz:hoff + (t + 1) * csz]
                    )
                    xt.append(xtile)

                def rhs(t, s, _xt=xt, _csz=csz):
                    return _xt[t][:, s * SUB:(s + 1) * SUB]
            else:
                xc = xpool.tile([KT, NKT * csz], BF16, tag="xc")
                nc.sync.dma_start(
                    xc[:], xH[:, hoff:hoff + NKT * csz]
                )

                def rhs(t, s, _xc=xc, _csz=csz):
                    return _xc[:, t * _csz + s * SUB:t * _csz + (s + 1) * SUB]

            for s in range(csz // SUB):
                hb = []
                for m in range(NMT):
                    ps = hps.tile([MT, SUB], F32, tag="ps")
                    for t in range(NKT):
                        nc.tensor.matmul(
                            ps[:],
                            w_sb(t, m),
                            rhs(t, s),
                            start=(t == 0),
                            stop=(t == NKT - 1),
                        )
                    h = hpool.tile([MT, SUB], BF16, tag=f"h{m}")
                    nc.scalar.activation(h[:], ps[:], relu, bias=b1_all[:, m:m + 1])
                    hb.append(h)
                flush_gemm2()  # previous sub's GEMM2 (relus are done)
                pending.append((hb, coff + s * SUB))
            coff += csz
        flush_gemm2()

    nc.compile()
    return nc


def _get_nc(b_shard: int = B_SHARD):
    if b_shard not in _CACHE:
        _CACHE[b_shard] = _build(b_shard)
    return _CACHE[b_shard]


def _host_prep(x, w_conv, w1, b1, w2, b2, b_shard=B_SHARD):
    """Fold conv into w1, pack weights, and lay out per-core inputs."""
    bf16 = ml_dtypes.bfloat16
    # Conv matrix Wc[784, 676]: feat[:, oi*26+oj] = sum_{di,dj} x[:, (oi+di)*28+(oj+dj)] * w_conv[di,dj]
    w_conv = np.asarray(w_conv, np.float64)
    oi = np.arange(26)
    oj = np.arange(26)
    wc = np.zeros((784, 676), np.float64)
    for di in range(3):
        for dj in range(3):
            src = ((oi[:, None] + di) * 28 + (oj[None, :] + dj)).ravel()
            dst = (oi[:, None] * 26 + oj[None, :]).ravel()
            wc[src, dst] += w_conv[di, dj]
    w_eff = (wc @ np.asarray(w1, np.float64)).astype(bf16)  # [784, 256]

    # per-(t,m) weight tiles: wt[t][m][p, j] = w_eff[t*KT+p, m*MT+j]
    wt = w_eff.reshape(NKT, KT, NMT, MT)
    w00 = np.ascontiguousarray(wt[0, :, 0, :])
    w0r = np.ascontiguousarray(
        wt[1:, :, 0, :].transpose(1, 0, 2).reshape(KT, -1)
    )
    w1a = np.ascontiguousarray(
        wt[:, :, 1, :].transpose(1, 0, 2).reshape(KT, -1)
    )
    # b1a[p, m] = b1[m*MT+p]
    b1a = np.ascontiguousarray(
        np.asarray(b1, np.float32).reshape(NMT, MT).T
    )
    # w2a[p, m*OUT_PAD + j] = w2_padded[m*MT+p, j]
    w2p = np.zeros((CH, OUT_PAD), bf16)
    w2p[:, :OUT_CH] = np.asarray(w2).astype(bf16)
    w2a = np.ascontiguousarray(
        w2p.reshape(NMT, MT, OUT_PAD).transpose(1, 0, 2).reshape(MT, -1)
    )
    b2c = np.zeros((OUT_PAD, 1), np.float32)
    b2c[:OUT_CH, 0] = np.asarray(b2, np.float32)

    x_bf = np.asarray(x).astype(bf16)  # [B, 784]
    in_maps = []
    for c in range(N_CORES):
        shard = x_bf[c * b_shard:(c + 1) * b_shard]  # [b_shard, 784]
        # chunk-major xH: for chunk (off, csz):
        #   xH[p, 7*off + t*csz + j] = shard[off + j, t*KT + p]
        pieces = []
        off = 0
        for csz in CHUNKS:
            blk = shard[off:off + csz, :].T            # [784, csz]
            blk = blk.reshape(NKT, KT, csz).transpose(1, 0, 2)
            pieces.append(blk.reshape(KT, NKT * csz))
            off += csz
        in_maps.append(
            {
                "xH": np.ascontiguousarray(np.concatenate(pieces, axis=1)),
                "w00": w00,
                "w0r": w0r,
                "w1a": w1a,
                "b1a": b1a,
                "w2a": w2a,
                "b2c": b2c,
            }
        )
    return in_maps


LAST_RESULT = None  # BassKernelResults of the most recent run (for test harness)


def kernel(x, w_conv, w1, b1, w2, b2):
    global LAST_RESULT
    nc = _get_nc()
    in_maps = _host_prep(x, w_conv, w1, b1, w2, b2)
    trace = bool(int(os.environ.get("KERNEL_TRACE", "0")))
    res = run_bass_kernel_spmd(
        nc, in_maps, list(range(N_CORES)), trace=trace,
        tmpdir=os.environ.get("KERNEL_TMPDIR") or None,
    )
    LAST_RESULT = res
    out = np.empty((B, OUT_CH), np.float32)
    for c in range(N_CORES):
        out[c * B_SHARD:(c + 1) * B_SHARD] = res.results[c]["outT"][:OUT_CH].T
    return out
